# revision 27
# baseline (speedup 1.0000x reference)
"""Multi-head causal self-attention (B=2, L=2048, D=1024, H=16) on 8 TRN2
NeuronCores.

Sharding: core c handles batch b = c // 4 and head group g = c % 4 (4 heads,
i.e. a 256-wide slice of the QKV output dim and the matching 256 rows of
Wo^T).  Each core computes a full (L, D) partial of the output projection;
the host sums the 4 partials per batch and adds bo.

Final design (166.5us HW, vs 315us baseline):
  - x and all weights are pre-transposed AND cast to f16 on the host, so
    x^T / W^T DMA straight into SBUF: no PE transposes, no DVE casts, half
    the input DMA bytes (f16 rounding ~1e-3 rel err; tolerance 2e-2).
  - phase A emits only what attention on the first q-block needs (Q, K for
    q-blocks 0-1, V group 0); K for blocks 2-3 and V groups 1-3 become
    "filler" chunks woven between attention pairs, so the projection work
    soaks up the PE slack while ACT (exp) is the pace-setter.
  - attention inner loop is software-pipelined one k-pair ahead of the PV
    matmuls so the PE never waits out the exp (ACT) latency.
  - causal diagonal tiles are column-truncated for qt>=1: scores/exp/PV
    only cover valid q columns (widths 512/384/256/128); the causal mask
    shrinks to one 128-col affine_select per diagonal tile.  Diagonal
    pairs run FIRST within a head so the final (full-width) off-diagonal
    PV carries the PSUM stop flag over the whole pso region.
  - softmax: ones-column of V accumulates the denominator into pso row 64;
    reciprocal_approx_fast (DVE) + gpsimd partition_broadcast + one DVE
    multiply normalize entirely off the PE.
  - weave scheduler priorities: softmax-normalize > projection fillers >
    output-projection units (held 2 slots after a norm so the outproj
    matmuls never wait on the just-emitted normalize chain).
  - PSUM: scores pairs 2x[128,1024] + pso 2x[128,512] + psy 1 + filler 1.
  - output partials are written as f16 (host upcasts and sums).
"""

import sys

for _p in ("/opt/trn_rl_repo", "/root/.axon_site/_ro/trn_rl_repo"):
    if _p not in sys.path:
        sys.path.append(_p)

from collections import deque
from contextlib import ExitStack

import numpy as np

import concourse.bass as bass
import concourse.tile as tile
from concourse import bacc, mybir
from concourse.bass_utils import run_bass_kernel_spmd

F32 = mybir.dt.float32
F32R = mybir.dt.float32r
F16 = mybir.dt.float16

B, L, D, H = 2, 2048, 1024, 16
DK = D // H  # 64
NCORES = 8
GH = 4  # heads per core
C = GH * DK  # 256: per-core slice of the qkv/head dim
QT_TILES = L // 512  # 4
KT_TILES = L // 128  # 16
DCH = D // 128  # 8


def _build_program():
    nc = bacc.Bacc("TRN2", target_bir_lowering=False, debug=False, num_devices=NCORES)

    xt_d = nc.dram_tensor("xt", [D, L], F16, kind="ExternalInput").ap()
    wqt_d = nc.dram_tensor("wqt", [D, C], F16, kind="ExternalInput").ap()
    wkt_d = nc.dram_tensor("wkt", [D, C], F16, kind="ExternalInput").ap()
    wvt_d = nc.dram_tensor("wvt", [D, C], F16, kind="ExternalInput").ap()
    wot_d = nc.dram_tensor("wot", [C, D], F16, kind="ExternalInput").ap()
    bq_d = nc.dram_tensor("bq", [C], F32, kind="ExternalInput").ap()
    bk_d = nc.dram_tensor("bk", [C], F32, kind="ExternalInput").ap()
    bv_d = nc.dram_tensor("bv", [4 * C], F16, kind="ExternalInput").ap()
    out_d = nc.dram_tensor("out", [L, D], F16, kind="ExternalOutput").ap()

    with tile.TileContext(nc) as tc, ExitStack() as ctx:
        pool = ctx.enter_context(tc.tile_pool(name="persist", bufs=1))

        ones_h = pool.tile([1, 128], F16)
        nc.gpsimd.memset(ones_h[:], 1.0)

        bq_sb = pool.tile([128, 2], F32)
        nc.sync.dma_start(bq_sb[:], bq_d.rearrange("(c p) -> p c", p=128))
        bk_sb = pool.tile([128, 2], F32)
        nc.sync.dma_start(bk_sb[:], bk_d.rearrange("(c p) -> p c", p=128))
        bv_sb = pool.tile([1, 4 * C], F16)

        QT = pool.tile([128, 2, L], F16)
        KTzs = [pool.tile([128, GH, 512], F16, name=f"KTz{g}") for g in range(4)]
        Vps = [pool.tile([128, 4, GH, DK + 1], F16, name=f"Vp{g}") for g in range(4)]
        OTs = [pool.tile([128, 2, 512], F16, name=f"OT{g}") for g in range(4)]
        WoT = pool.tile([128, 2, D], F16)
        MASK = pool.tile([128, 896], F16)

        pp = ctx.enter_context(tc.tile_pool(name="projs", bufs=1))
        XT = pp.tile([128, DCH, L], F16)
        WT = {}
        for name in ("q", "k", "v"):
            WT[name] = pp.tile([128, DCH, C], F16, name=f"W{name}T")

        with nc.allow_low_precision(reason="f16 activations/weights; f32 psum"):
            # ---- input DMA over three queues (SP + ACT HW DGE, Pool SW) ----
            # x chunks: evens on SP, c1/c3 trail Wq on ACT, c5/c7 on Pool
            for c in range(DCH):
                nc.scalar.dma_start(WT["q"][:, c, :], wqt_d[c * 128 : (c + 1) * 128, :])
            for c in (5, 7):
                nc.gpsimd.dma_start(XT[:, c, :], xt_d[c * 128 : (c + 1) * 128, :])
            for c in (0, 2, 4, 6):
                nc.sync.dma_start(XT[:, c, :], xt_d[c * 128 : (c + 1) * 128, :])
            for c in (1, 3):
                nc.scalar.dma_start(XT[:, c, :], xt_d[c * 128 : (c + 1) * 128, :])
            for name, w_d in (("k", wkt_d), ("v", wvt_d)):
                for c in range(DCH):
                    eng = nc.sync if c % 2 == 1 else nc.scalar
                    eng.dma_start(WT[name][:, c, :], w_d[c * 128 : (c + 1) * 128, :])
            nc.sync.dma_start(bv_sb[:], bv_d[None, :])
            for c in range(2):
                nc.sync.dma_start(WoT[:, c, :], wot_d[c * 128 : (c + 1) * 128, :])
            # memsets AFTER the Pool-queue DMA issues (in-order queue)
            for g in range(4):
                nc.gpsimd.memset(KTzs[g][:], 0.0)
            for g in range(4):
                nc.gpsimd.memset(Vps[g][:, :, :, DK : DK + 1], 1.0)
            # causal mask tile: MASK[k, j] = 1 iff j >= k + 384.  Slicing at
            # s = 384 - 128*kt gives "keep iff c >= k + 128*kt" for width-512
            # qt=0 tiles; [384:512] is the generic 128-col triangle.
            nc.gpsimd.memset(MASK[:], 1.0)
            nc.gpsimd.affine_select(
                out=MASK[:],
                in_=MASK[:],
                pattern=[[1, 896]],
                compare_op=mybir.AluOpType.is_ge,
                fill=0.0,
                base=-384,
                channel_multiplier=-1,
            )

            def k_bias_writes(qt, j, ps, qh=None, width=512):
                """KTz[qt] <- ps halves + bias (ps is [128, 512] slice view)."""
                for half in range(2):
                    hp = 64 * half
                    nc.vector.tensor_tensor(
                        KTzs[qt][hp : hp + 64, 2 * j + half, :],
                        ps[hp : hp + 64, :],
                        bk_sb[hp : hp + 64, j, None].to_broadcast((64, 512)),
                        mybir.AluOpType.add,
                    )

            # ======== phase A part 1 (scoped PSUM): Q, K(qt 0-1), V g0 ====
            with tc.tile_pool(name="psA", bufs=4, space="PSUM") as psA:
                # ---- Q: dci-outer over 4 [128,1024] accumulators ----
                acc = [
                    psA.tile([128, 1024], F32, tag="psA", name=f"psq{i}")
                    for i in range(4)
                ]
                # consume x chunks in DMA-arrival order: evens stream on the
                # sync queue immediately, odds trail Wq on the scalar queue
                dci_order = [0, 5, 2, 1, 7, 4, 3, 6]
                for di, dci in enumerate(dci_order):
                    for j in range(2):
                        for qp in range(2):
                            for hf in range(2):
                                nc.tensor.matmul(
                                    acc[2 * j + qp][:, hf * 512 : (hf + 1) * 512],
                                    lhsT=WT["q"][:, dci, j * 128 : (j + 1) * 128],
                                    rhs=XT[
                                        :,
                                        dci,
                                        qp * 1024 + hf * 512 : qp * 1024 + (hf + 1) * 512,
                                    ],
                                    start=(di == 0),
                                    stop=(di == DCH - 1),
                                )
                for j in range(2):
                    for qp in range(2):
                        nc.vector.tensor_tensor(
                            QT[:, j, qp * 1024 : (qp + 1) * 1024],
                            acc[2 * j + qp][:],
                            bq_sb[:, j, None].to_broadcast((128, 1024)),
                            mybir.AluOpType.add,
                        )

                # ---- K for q-blocks 0-1 (qp=0): 2 accumulators ----
                acck = [
                    psA.tile([128, 1024], F32, tag="psA", name=f"psk{j}")
                    for j in range(2)
                ]
                for dci in range(DCH):
                    for j in range(2):
                        for hf in range(2):
                            nc.tensor.matmul(
                                acck[j][:, hf * 512 : (hf + 1) * 512],
                                lhsT=WT["k"][:, dci, j * 128 : (j + 1) * 128],
                                rhs=XT[:, dci, hf * 512 : (hf + 1) * 512],
                                start=(dci == 0),
                                stop=(dci == DCH - 1),
                            )
                for j in range(2):
                    for qh in range(2):
                        k_bias_writes(qh, j, acck[j][:, qh * 512 : (qh + 1) * 512])

            # ======== attention pools (PSUM freed by psA close) ========
            pssP = ctx.enter_context(tc.tile_pool(name="pss", bufs=2, space="PSUM"))
            psoP = ctx.enter_context(tc.tile_pool(name="pso", bufs=2, space="PSUM"))
            psyP = ctx.enter_context(tc.tile_pool(name="psy", bufs=1, space="PSUM"))
            pfill = ctx.enter_context(tc.tile_pool(name="pfill", bufs=1, space="PSUM"))
            cp = ctx.enter_context(tc.tile_pool(name="copies", bufs=3))
            yp = ctx.enter_context(tc.tile_pool(name="youts", bufs=3))

            # ---- filler chunks: K(qt 2-3), V groups 1-3, woven into attention
            def k_chunk(qt, j):
                def fn():
                    ps = pfill.tile([128, 512], F32, tag="pf")
                    for dci in range(DCH):
                        nc.tensor.matmul(
                            ps[:],
                            lhsT=WT["k"][:, dci, j * 128 : (j + 1) * 128],
                            rhs=XT[:, dci, qt * 512 : (qt + 1) * 512],
                            start=(dci == 0),
                            stop=(dci == DCH - 1),
                        )
                    k_bias_writes(qt, j, ps[:])

                return fn

            def v_chunk(g, kp):
                def fn():
                    ps = pfill.tile([128, 512], F32, tag="pf")
                    for ksub in (2 * kp, 2 * kp + 1):
                        kt = 4 * g + ksub
                        for dci in range(DCH):
                            nc.tensor.matmul(
                                ps[:, (ksub % 2) * 256 : (ksub % 2 + 1) * 256],
                                lhsT=XT[:, dci, kt * 128 : (kt + 1) * 128],
                                rhs=WT["v"][:, dci, :],
                                start=(dci == 0 and ksub % 2 == 0),
                                stop=False,
                            )
                    nc.tensor.matmul(
                        ps[:], lhsT=ones_h[:], rhs=bv_sb[:, 0:512], start=False, stop=True
                    )
                    nc.vector.tensor_copy(
                        Vps[g][:, 2 * kp : 2 * kp + 2, :, 0:DK],
                        ps[:].rearrange("p (k h d) -> p k h d", k=2, h=GH),
                    )

                return fn

            dfill = deque()
            dfill.append((0, v_chunk(0, 1)))
            for kp in range(2):
                dfill.append((1, v_chunk(1, kp)))
            for j in range(2):
                dfill.append((2, k_chunk(2, j)))
            for kp in range(2):
                dfill.append((2, v_chunk(2, kp)))
            for j in range(2):
                dfill.append((3, k_chunk(3, j)))
            for kp in range(2):
                dfill.append((3, v_chunk(3, kp)))

            dnorm = deque()
            dnorm_b = deque()
            dproj = deque()
            proj_hold = [0]
            slot = [0]

            def normalize_a(h, qt, pso):
                den_sb = cp.tile([1, 512], F32, tag="den", bufs=2)
                nc.vector.tensor_copy(den_sb[:], pso[64:65, :])
                rden = cp.tile([1, 512], F32, tag="rden", bufs=2)
                nc.vector.reciprocal_approx_fast(rden[:], den_sb[:])
                rb = cp.tile([64, 512], F32, tag="rb", bufs=2)
                nc.gpsimd.partition_broadcast(rb[:], rden[:], channels=64)
                return rb

            def normalize_b(h, qt, pso, rb, split=False):
                hj, hp = h // 2, 64 * (h % 2)
                if split:
                    for sub in range(4):
                        s = slice(sub * 128, (sub + 1) * 128)
                        nc.vector.tensor_tensor(
                            OTs[qt][hp : hp + 64, hj, s],
                            pso[:64, s],
                            rb[:, s],
                            mybir.AluOpType.mult,
                        )
                else:
                    nc.vector.tensor_tensor(
                        OTs[qt][hp : hp + 64, hj, :],
                        pso[:64],
                        rb[:],
                        mybir.AluOpType.mult,
                    )

            tail_alt = [0]

            def outproj_unit(qt512, sub, e, on_act=False):
                q0 = qt512 * 512 + sub * 128
                if on_act:
                    # tail: ping-pong across the psy and (now idle) filler
                    # pools so consecutive units overlap in PSUM
                    tail_alt[0] ^= 1
                    pl = psyP if tail_alt[0] else pfill
                    psy = pl.tile([128, 512], F32, tag="psy" if tail_alt[0] else "pf")
                else:
                    psy = psyP.tile([128, 512], F32, tag="psy")
                for cj in range(2):
                    nc.tensor.matmul(
                        psy[:],
                        lhsT=OTs[qt512][:, cj, sub * 128 : (sub + 1) * 128],
                        rhs=WoT[:, cj, e * 512 : (e + 1) * 512],
                        start=(cj == 0),
                        stop=(cj == 1),
                    )
                y_sb = yp.tile([128, 512], F16, tag="y")
                if on_act and tail_alt[0]:
                    # tail only: ACT is idle there ('copy' shares the exp
                    # table); in-window this would block later exps.  The
                    # other half stays on DVE so both engines drain psum.
                    nc.scalar.activation(
                        y_sb[:], psy[:], mybir.ActivationFunctionType.Copy
                    )
                else:
                    nc.vector.tensor_copy(y_sb[:], psy[:])
                nc.sync.dma_start(
                    out_d[q0 : q0 + 128, e * 512 : (e + 1) * 512], y_sb[:]
                )

            def pop_deferred(tail=False):
                # norm_a launches the den->recip->broadcast chain; norm_b (the
                # OT multiply, which WAITS on the broadcast) runs >=2 slots
                # later so it never blocks the DVE queue head.  Fillers are PE
                # work that hides chain latency.  Outproj units wait two slots
                # after a norm_b so their matmuls don't stall on OT.
                slot[0] += 1
                if dnorm:
                    args = dnorm.popleft()
                    rb = normalize_a(*args)
                    dnorm_b.append((slot[0] + 2, args, rb))
                    return
                if dnorm_b and (tail or dnorm_b[0][0] <= slot[0]):
                    _, args, rb = dnorm_b.popleft()
                    normalize_b(*args, rb, split=tail)
                    proj_hold[0] = 2
                    return
                if dfill:
                    _, fn = dfill.popleft()
                    fn()
                    proj_hold[0] = max(0, proj_hold[0] - 1)
                    return
                if proj_hold[0] > 0 and not tail:
                    proj_hold[0] -= 1
                    return
                if dproj:
                    outproj_unit(*dproj.popleft(), on_act=tail)

            def flush_fillers(upto_qt):
                while dfill and dfill[0][0] <= upto_qt:
                    _, fn = dfill.popleft()
                    fn()

            pend_pv = None  # (pso, p_sb, entries, h, qt, is_last)

            def emit_pv(state):
                pso, p_sb, entries, h, qt, is_last = state
                for kt, w, qoff, poff, st, sp in entries:
                    nc.tensor.matmul(
                        pso[:65, qoff : qoff + w],
                        lhsT=Vps[kt // 4][:, kt % 4, h, :],
                        rhs=p_sb[:, poff : poff + w],
                        start=st,
                        stop=sp,
                    )
                if is_last:
                    dnorm.append((h, qt, pso))
                    if h == GH - 1:
                        for sub in range(4):
                            for e in range(2):
                                dproj.append((qt, sub, e))

            v_chunk(0, 0)()  # kts 0-1: needed by the very first PV pair

            for qt in range(QT_TILES):
                if qt > 0:
                    flush_fillers(qt)
                # pair list: entries (kt, width, qoff, poff); diagonal first
                if qt == 0:
                    pairs = [
                        [(0, 512, 0, 0), (1, 512, 0, 512)],
                        [(2, 512, 0, 0), (3, 512, 0, 512)],
                    ]
                    full_mask = True
                else:
                    d0 = 4 * qt
                    pairs = [
                        [(d0, 512, 0, 0), (d0 + 1, 384, 128, 512)],
                        [(d0 + 2, 256, 256, 0), (d0 + 3, 128, 384, 256)],
                    ]
                    for m in range(2 * qt):
                        pairs.append(
                            [(2 * m, 512, 0, 0), (2 * m + 1, 512, 0, 512)]
                        )
                    full_mask = False
                n_pairs = len(pairs)
                for h in range(GH):
                    pso = psoP.tile([128, 512], F32, tag="pso")
                    first_pv = True
                    for pi, pair in enumerate(pairs):
                        totw = sum(p[1] for p in pair)
                        pss = pssP.tile([128, 1024], F32, tag="pss")
                        for kt, w, qoff, poff in pair:
                            nc.tensor.matmul(
                                pss[:, poff : poff + w],
                                lhsT=KTzs[kt // 4][
                                    :, h, (kt % 4) * 128 : (kt % 4 + 1) * 128
                                ],
                                rhs=QT[:, h // 2, qt * 512 + qoff : qt * 512 + qoff + w],
                                start=True,
                                stop=True,
                            )
                        p_sb = cp.tile([128, 1024], F16, tag="p", bufs=5)
                        nc.scalar.activation(
                            p_sb[:, 0:totw],
                            pss[:, 0:totw],
                            mybir.ActivationFunctionType.Exp,
                            scale=0.125,
                        )
                        for kt, w, qoff, poff in pair:
                            if kt < 4 * qt:
                                continue  # off-diagonal: no mask
                            if full_mask:
                                s = 384 - 128 * kt
                                nc.vector.tensor_tensor(
                                    p_sb[:, poff : poff + w],
                                    p_sb[:, poff : poff + w],
                                    MASK[:, s : s + w],
                                    mybir.AluOpType.mult,
                                )
                            else:
                                # truncated: only first 128 cols can violate
                                nc.vector.tensor_tensor(
                                    p_sb[:, poff : poff + 128],
                                    p_sb[:, poff : poff + 128],
                                    MASK[:, 384:512],
                                    mybir.AluOpType.mult,
                                )
                        if pend_pv is not None:
                            emit_pv(pend_pv)
                            pop_deferred()
                        entries = []
                        for kt, w, qoff, poff in pair:
                            entries.append(
                                (
                                    kt,
                                    w,
                                    qoff,
                                    poff,
                                    first_pv,
                                    pi == n_pairs - 1 and kt == pair[-1][0],
                                )
                            )
                            first_pv = False
                        pend_pv = (pso, p_sb, entries, h, qt, pi == n_pairs - 1)
            emit_pv(pend_pv)
            while dnorm or dnorm_b or dfill or dproj:
                pop_deferred(tail=True)

    nc.compile()
    return nc


_NC_CACHE = None


def _get_program():
    global _NC_CACHE
    if _NC_CACHE is None:
        _NC_CACHE = _build_program()
    return _NC_CACHE


def _run(in_maps, trace=False, **kw):
    nc = _get_program()
    return run_bass_kernel_spmd(nc, in_maps, list(range(NCORES)), trace=trace, **kw)


def _t16(a):
    return np.ascontiguousarray(np.asarray(a).T.astype(np.float16, order="C"))


def _make_in_maps(x, Wq, bq, Wk, bk, Wv, bv, Wo, bo):
    xts = [_t16(np.asarray(x)[b]) for b in range(B)]  # [D, L] f16
    in_maps = []
    for core in range(NCORES):
        b, g = divmod(core, 4)
        s = slice(g * C, (g + 1) * C)
        in_maps.append(
            {
                "xt": xts[b],
                "wqt": _t16(np.asarray(Wq)[s, :]),  # [D, C]
                "wkt": _t16(np.asarray(Wk)[s, :]),
                "wvt": _t16(np.asarray(Wv)[s, :]),
                "wot": _t16(np.asarray(Wo)[:, s]),  # [C, D]
                "bq": np.ascontiguousarray(np.asarray(bq)[s], dtype=np.float32),
                "bk": np.ascontiguousarray(np.asarray(bk)[s], dtype=np.float32),
                "bv": np.tile(np.asarray(bv)[s].astype(np.float16), 4),
            }
        )
    return in_maps


def kernel(x, Wq, bq, Wk, bk, Wv, bv, Wo, bo, _trace=False, _trace_out=None, _tmpdir=None):
    in_maps = _make_in_maps(x, Wq, bq, Wk, bk, Wv, bv, Wo, bo)
    res = _run(in_maps, trace=_trace, tmpdir=_tmpdir)
    if _trace_out is not None:
        _trace_out.append(res)
    bo = np.asarray(bo, dtype=np.float32)
    out = np.empty((B, L, D), dtype=np.float32)
    for b in range(B):
        acc = res.results[4 * b]["out"].astype(np.float32)
        for g in range(1, 4):
            acc = acc + res.results[4 * b + g]["out"].astype(np.float32)
        out[b] = acc + bo[None, :]
    return out


# revision 29
# speedup vs baseline: 1.0295x; 1.0295x over previous
"""Multi-head causal self-attention (B=2, L=2048, D=1024, H=16) on 8 TRN2
NeuronCores.

Sharding: core c handles batch b = c // 4 and head group g = c % 4 (4 heads,
i.e. a 256-wide slice of the QKV output dim and the matching 256 rows of
Wo^T).  Each core computes a full (L, D) partial of the output projection;
the host sums the 4 partials per batch and adds bo.

Final design (166.5us HW, vs 315us baseline):
  - x and all weights are pre-transposed AND cast to f16 on the host, so
    x^T / W^T DMA straight into SBUF: no PE transposes, no DVE casts, half
    the input DMA bytes (f16 rounding ~1e-3 rel err; tolerance 2e-2).
  - phase A emits only what attention on the first q-block needs (Q, K for
    q-blocks 0-1, V group 0); K for blocks 2-3 and V groups 1-3 become
    "filler" chunks woven between attention pairs, so the projection work
    soaks up the PE slack while ACT (exp) is the pace-setter.
  - attention inner loop is software-pipelined one k-pair ahead of the PV
    matmuls so the PE never waits out the exp (ACT) latency.
  - causal diagonal tiles are column-truncated for qt>=1: scores/exp/PV
    only cover valid q columns (widths 512/384/256/128); the causal mask
    shrinks to one 128-col affine_select per diagonal tile.  Diagonal
    pairs run FIRST within a head so the final (full-width) off-diagonal
    PV carries the PSUM stop flag over the whole pso region.
  - softmax: ones-column of V accumulates the denominator into pso row 64;
    reciprocal_approx_fast (DVE) + gpsimd partition_broadcast + one DVE
    multiply normalize entirely off the PE.
  - weave scheduler priorities: softmax-normalize > projection fillers >
    output-projection units (held 2 slots after a norm so the outproj
    matmuls never wait on the just-emitted normalize chain).
  - PSUM: scores pairs 2x[128,1024] + pso 2x[128,512] + psy 1 + filler 1.
  - output partials are written as f16 (host upcasts and sums).
"""

import sys

for _p in ("/opt/trn_rl_repo", "/root/.axon_site/_ro/trn_rl_repo"):
    if _p not in sys.path:
        sys.path.append(_p)

from collections import deque
from contextlib import ExitStack

import numpy as np

import concourse.bass as bass
import concourse.tile as tile
from concourse import bacc, mybir
from concourse.bass_utils import run_bass_kernel_spmd

F32 = mybir.dt.float32
F32R = mybir.dt.float32r
F16 = mybir.dt.float16

B, L, D, H = 2, 2048, 1024, 16
DK = D // H  # 64
NCORES = 8
GH = 4  # heads per core
C = GH * DK  # 256: per-core slice of the qkv/head dim
QT_TILES = L // 512  # 4
KT_TILES = L // 128  # 16
DCH = D // 128  # 8


def _build_program():
    nc = bacc.Bacc("TRN2", target_bir_lowering=False, debug=False, num_devices=NCORES)

    xt_d = nc.dram_tensor("xt", [D, L], F16, kind="ExternalInput").ap()
    wqt_d = nc.dram_tensor("wqt", [D, C], F16, kind="ExternalInput").ap()
    wkt_d = nc.dram_tensor("wkt", [D, C], F16, kind="ExternalInput").ap()
    wvt_d = nc.dram_tensor("wvt", [D, C], F16, kind="ExternalInput").ap()
    wot_d = nc.dram_tensor("wot", [C, D], F16, kind="ExternalInput").ap()
    bq_d = nc.dram_tensor("bq", [C], F32, kind="ExternalInput").ap()
    bk_d = nc.dram_tensor("bk", [C], F32, kind="ExternalInput").ap()
    bv_d = nc.dram_tensor("bv", [4 * C], F16, kind="ExternalInput").ap()
    out_d = nc.dram_tensor("out", [L, D], F16, kind="ExternalOutput").ap()

    with tile.TileContext(nc) as tc, ExitStack() as ctx:
        pool = ctx.enter_context(tc.tile_pool(name="persist", bufs=1))

        ones_h = pool.tile([1, 128], F16)
        nc.gpsimd.memset(ones_h[:], 1.0)

        bq_sb = pool.tile([128, 2], F32)
        nc.sync.dma_start(bq_sb[:], bq_d.rearrange("(c p) -> p c", p=128))
        bk_sb = pool.tile([128, 2], F32)
        nc.sync.dma_start(bk_sb[:], bk_d.rearrange("(c p) -> p c", p=128))
        bv_sb = pool.tile([1, 4 * C], F16)

        QT = pool.tile([128, 2, L], F16)
        KTzs = [pool.tile([128, GH, 512], F16, name=f"KTz{g}") for g in range(4)]
        Vps = [pool.tile([128, 4, GH, DK + 1], F16, name=f"Vp{g}") for g in range(4)]
        OTs = [pool.tile([128, 2, 512], F16, name=f"OT{g}") for g in range(4)]
        WoT = pool.tile([128, 2, D], F16)
        MASK = pool.tile([128, 896], F16)

        pp = ctx.enter_context(tc.tile_pool(name="projs", bufs=1))
        XT = pp.tile([128, DCH, L], F16)
        WT = {}
        for name in ("q", "k", "v"):
            WT[name] = pp.tile([128, DCH, C], F16, name=f"W{name}T")

        with nc.allow_low_precision(reason="f16 activations/weights; f32 psum"):
            # ---- input DMA over three queues (SP + ACT HW DGE, Pool SW) ----
            # x chunks: evens on SP, c1/c3 trail Wq on ACT, c5/c7 on Pool
            for c in range(DCH):
                nc.scalar.dma_start(WT["q"][:, c, :], wqt_d[c * 128 : (c + 1) * 128, :])
            for c in (5, 7):
                nc.gpsimd.dma_start(XT[:, c, :], xt_d[c * 128 : (c + 1) * 128, :])
            for c in (0, 2, 4, 6):
                nc.sync.dma_start(XT[:, c, :], xt_d[c * 128 : (c + 1) * 128, :])
            for c in (1, 3):
                nc.scalar.dma_start(XT[:, c, :], xt_d[c * 128 : (c + 1) * 128, :])
            for name, w_d in (("k", wkt_d), ("v", wvt_d)):
                for c in range(DCH):
                    eng = nc.sync if c % 2 == 1 else nc.scalar
                    eng.dma_start(WT[name][:, c, :], w_d[c * 128 : (c + 1) * 128, :])
            nc.sync.dma_start(bv_sb[:], bv_d[None, :])
            for c in range(2):
                nc.sync.dma_start(WoT[:, c, :], wot_d[c * 128 : (c + 1) * 128, :])
            # memsets AFTER the Pool-queue DMA issues (in-order queue)
            for g in range(4):
                nc.gpsimd.memset(KTzs[g][:], 0.0)
            for g in range(4):
                nc.gpsimd.memset(Vps[g][:, :, :, DK : DK + 1], 1.0)
            # causal mask tile: MASK[k, j] = 1 iff j >= k + 384.  Slicing at
            # s = 384 - 128*kt gives "keep iff c >= k + 128*kt" for width-512
            # qt=0 tiles; [384:512] is the generic 128-col triangle.
            nc.gpsimd.memset(MASK[:], 1.0)
            nc.gpsimd.affine_select(
                out=MASK[:],
                in_=MASK[:],
                pattern=[[1, 896]],
                compare_op=mybir.AluOpType.is_ge,
                fill=0.0,
                base=-384,
                channel_multiplier=-1,
            )

            def k_bias_writes(qt, j, ps, qh=None, width=512):
                """KTz[qt] <- ps halves + bias (ps is [128, 512] slice view)."""
                for half in range(2):
                    hp = 64 * half
                    nc.vector.tensor_tensor(
                        KTzs[qt][hp : hp + 64, 2 * j + half, :],
                        ps[hp : hp + 64, :],
                        bk_sb[hp : hp + 64, j, None].to_broadcast((64, 512)),
                        mybir.AluOpType.add,
                    )

            # ======== phase A part 1 (scoped PSUM): Q, K(qt 0-1), V g0 ====
            with tc.tile_pool(name="psA", bufs=4, space="PSUM") as psA:
                # ---- Q: dci-outer over 4 [128,1024] accumulators ----
                acc = [
                    psA.tile([128, 1024], F32, tag="psA", name=f"psq{i}")
                    for i in range(4)
                ]
                # consume x chunks in DMA-arrival order: evens stream on the
                # sync queue immediately, odds trail Wq on the scalar queue
                dci_order = [0, 5, 2, 1, 7, 4, 3, 6]
                for di, dci in enumerate(dci_order):
                    for j in range(2):
                        for qp in range(2):
                            for hf in range(2):
                                nc.tensor.matmul(
                                    acc[2 * j + qp][:, hf * 512 : (hf + 1) * 512],
                                    lhsT=WT["q"][:, dci, j * 128 : (j + 1) * 128],
                                    rhs=XT[
                                        :,
                                        dci,
                                        qp * 1024 + hf * 512 : qp * 1024 + (hf + 1) * 512,
                                    ],
                                    start=(di == 0),
                                    stop=(di == DCH - 1),
                                )
                for j in range(2):
                    for qp in range(2):
                        nc.vector.tensor_tensor(
                            QT[:, j, qp * 1024 : (qp + 1) * 1024],
                            acc[2 * j + qp][:],
                            bq_sb[:, j, None].to_broadcast((128, 1024)),
                            mybir.AluOpType.add,
                        )

                # ---- K for q-blocks 0-1 (qp=0): 2 accumulators ----
                acck = [
                    psA.tile([128, 1024], F32, tag="psA", name=f"psk{j}")
                    for j in range(2)
                ]
                for dci in range(DCH):
                    for j in range(2):
                        for hf in range(2):
                            nc.tensor.matmul(
                                acck[j][:, hf * 512 : (hf + 1) * 512],
                                lhsT=WT["k"][:, dci, j * 128 : (j + 1) * 128],
                                rhs=XT[:, dci, hf * 512 : (hf + 1) * 512],
                                start=(dci == 0),
                                stop=(dci == DCH - 1),
                            )
                for j in range(2):
                    for qh in range(2):
                        k_bias_writes(qh, j, acck[j][:, qh * 512 : (qh + 1) * 512])

                # ---- V group 0 (kt 0-3) ----
                psv = psA.tile([128, 1024], F32, tag="psA", name="psV0")
                for ksub in range(4):
                    kt = ksub
                    for dci in range(DCH):
                        nc.tensor.matmul(
                            psv[:, ksub * 256 : (ksub + 1) * 256],
                            lhsT=XT[:, dci, kt * 128 : (kt + 1) * 128],
                            rhs=WT["v"][:, dci, :],
                            start=(dci == 0 and ksub % 2 == 0),
                            stop=False,
                        )
                for hf in range(2):
                    nc.tensor.matmul(
                        psv[:, hf * 512 : (hf + 1) * 512],
                        lhsT=ones_h[:],
                        rhs=bv_sb[:, hf * 512 : (hf + 1) * 512],
                        start=False,
                        stop=True,
                    )
                for hf in range(2):
                    nc.vector.tensor_copy(
                        Vps[0][:, 2 * hf : 2 * hf + 2, :, 0:DK],
                        psv[:, hf * 512 : (hf + 1) * 512].rearrange(
                            "p (k h d) -> p k h d", k=2, h=GH
                        ),
                    )

            # ======== attention pools (PSUM freed by psA close) ========
            pssP = ctx.enter_context(tc.tile_pool(name="pss", bufs=2, space="PSUM"))
            psoP = ctx.enter_context(tc.tile_pool(name="pso", bufs=2, space="PSUM"))
            psyP = ctx.enter_context(tc.tile_pool(name="psy", bufs=1, space="PSUM"))
            pfill = ctx.enter_context(tc.tile_pool(name="pfill", bufs=1, space="PSUM"))
            cp = ctx.enter_context(tc.tile_pool(name="copies", bufs=3))
            yp = ctx.enter_context(tc.tile_pool(name="youts", bufs=3))

            # ---- filler chunks: K(qt 2-3), V groups 1-3, woven into attention
            def k_chunk(qt, j):
                def fn():
                    ps = pfill.tile([128, 512], F32, tag="pf")
                    for dci in range(DCH):
                        nc.tensor.matmul(
                            ps[:],
                            lhsT=WT["k"][:, dci, j * 128 : (j + 1) * 128],
                            rhs=XT[:, dci, qt * 512 : (qt + 1) * 512],
                            start=(dci == 0),
                            stop=(dci == DCH - 1),
                        )
                    k_bias_writes(qt, j, ps[:])

                return fn

            def v_chunk(g, kp):
                def fn():
                    ps = pfill.tile([128, 512], F32, tag="pf")
                    for ksub in (2 * kp, 2 * kp + 1):
                        kt = 4 * g + ksub
                        for dci in range(DCH):
                            nc.tensor.matmul(
                                ps[:, (ksub % 2) * 256 : (ksub % 2 + 1) * 256],
                                lhsT=XT[:, dci, kt * 128 : (kt + 1) * 128],
                                rhs=WT["v"][:, dci, :],
                                start=(dci == 0 and ksub % 2 == 0),
                                stop=False,
                            )
                    nc.tensor.matmul(
                        ps[:], lhsT=ones_h[:], rhs=bv_sb[:, 0:512], start=False, stop=True
                    )
                    nc.vector.tensor_copy(
                        Vps[g][:, 2 * kp : 2 * kp + 2, :, 0:DK],
                        ps[:].rearrange("p (k h d) -> p k h d", k=2, h=GH),
                    )

                return fn

            dfill = deque()
            for kp in range(2):
                dfill.append((1, v_chunk(1, kp)))
            for j in range(2):
                dfill.append((2, k_chunk(2, j)))
            for kp in range(2):
                dfill.append((2, v_chunk(2, kp)))
            for j in range(2):
                dfill.append((3, k_chunk(3, j)))
            for kp in range(2):
                dfill.append((3, v_chunk(3, kp)))

            dnorm = deque()
            dnorm_b = deque()
            dproj = deque()
            proj_hold = [0]
            slot = [0]

            def normalize_a(h, qt, pso):
                den_sb = cp.tile([1, 512], F32, tag="den", bufs=2)
                nc.vector.tensor_copy(den_sb[:], pso[64:65, :])
                rden = cp.tile([1, 512], F32, tag="rden", bufs=2)
                nc.vector.reciprocal_approx_fast(rden[:], den_sb[:])
                rb = cp.tile([64, 512], F32, tag="rb", bufs=2)
                nc.gpsimd.partition_broadcast(rb[:], rden[:], channels=64)
                return rb

            def normalize_b(h, qt, pso, rb, split=False):
                hj, hp = h // 2, 64 * (h % 2)
                if split:
                    for sub in range(4):
                        s = slice(sub * 128, (sub + 1) * 128)
                        nc.vector.tensor_tensor(
                            OTs[qt][hp : hp + 64, hj, s],
                            pso[:64, s],
                            rb[:, s],
                            mybir.AluOpType.mult,
                        )
                else:
                    nc.vector.tensor_tensor(
                        OTs[qt][hp : hp + 64, hj, :],
                        pso[:64],
                        rb[:],
                        mybir.AluOpType.mult,
                    )

            tail_alt = [0]

            def outproj_unit(qt512, sub, e, on_act=False):
                q0 = qt512 * 512 + sub * 128
                if on_act:
                    # tail: ping-pong across the psy and (now idle) filler
                    # pools so consecutive units overlap in PSUM
                    tail_alt[0] ^= 1
                    pl = psyP if tail_alt[0] else pfill
                    psy = pl.tile([128, 512], F32, tag="psy" if tail_alt[0] else "pf")
                else:
                    psy = psyP.tile([128, 512], F32, tag="psy")
                for cj in range(2):
                    nc.tensor.matmul(
                        psy[:],
                        lhsT=OTs[qt512][:, cj, sub * 128 : (sub + 1) * 128],
                        rhs=WoT[:, cj, e * 512 : (e + 1) * 512],
                        start=(cj == 0),
                        stop=(cj == 1),
                    )
                y_sb = yp.tile([128, 512], F16, tag="y")
                if on_act and tail_alt[0]:
                    # tail only: ACT is idle there ('copy' shares the exp
                    # table); in-window this would block later exps.  The
                    # other half stays on DVE so both engines drain psum.
                    nc.scalar.activation(
                        y_sb[:], psy[:], mybir.ActivationFunctionType.Copy
                    )
                else:
                    nc.vector.tensor_copy(y_sb[:], psy[:])
                nc.sync.dma_start(
                    out_d[q0 : q0 + 128, e * 512 : (e + 1) * 512], y_sb[:]
                )

            def pop_deferred(tail=False):
                # norm_a launches the den->recip->broadcast chain; norm_b (the
                # OT multiply, which WAITS on the broadcast) runs >=2 slots
                # later so it never blocks the DVE queue head.  Fillers are PE
                # work that hides chain latency.  Outproj units wait two slots
                # after a norm_b so their matmuls don't stall on OT.
                slot[0] += 1
                if dnorm:
                    args = dnorm.popleft()
                    rb = normalize_a(*args)
                    dnorm_b.append((slot[0] + 2, args, rb))
                    return
                if dnorm_b and (tail or dnorm_b[0][0] <= slot[0]):
                    _, args, rb = dnorm_b.popleft()
                    normalize_b(*args, rb, split=tail)
                    proj_hold[0] = 2
                    return
                if dfill:
                    _, fn = dfill.popleft()
                    fn()
                    proj_hold[0] = max(0, proj_hold[0] - 1)
                    return
                if proj_hold[0] > 0 and not tail:
                    proj_hold[0] -= 1
                    return
                if dproj:
                    outproj_unit(*dproj.popleft(), on_act=tail)

            def flush_fillers(upto_qt):
                while dfill and dfill[0][0] <= upto_qt:
                    _, fn = dfill.popleft()
                    fn()

            pend_pv = None  # (pso, p_sb, entries, h, qt, is_last)

            def emit_pv(state):
                pso, p_sb, entries, h, qt, is_last = state
                for kt, w, qoff, poff, st, sp in entries:
                    nc.tensor.matmul(
                        pso[:65, qoff : qoff + w],
                        lhsT=Vps[kt // 4][:, kt % 4, h, :],
                        rhs=p_sb[:, poff : poff + w],
                        start=st,
                        stop=sp,
                    )
                if is_last:
                    dnorm.append((h, qt, pso))
                    if h == GH - 1:
                        for sub in range(4):
                            for e in range(2):
                                dproj.append((qt, sub, e))

            for qt in range(QT_TILES):
                flush_fillers(qt)
                # pair list: entries (kt, width, qoff, poff); diagonal first.
                # qt=0 truncates too: the PSUM stop flag clears the whole
                # 2KB zero-region it touches, so the partial-width final PV
                # (kt3, [384:512)) still closes the bank's accumulation group.
                if qt == 0:
                    pairs = [
                        [(0, 512, 0, 0), (1, 384, 128, 512)],
                        [(2, 256, 256, 0), (3, 128, 384, 256)],
                    ]
                    full_mask = False
                else:
                    d0 = 4 * qt
                    pairs = [
                        [(d0, 512, 0, 0), (d0 + 1, 384, 128, 512)],
                        [(d0 + 2, 256, 256, 0), (d0 + 3, 128, 384, 256)],
                    ]
                    for m in range(2 * qt):
                        pairs.append(
                            [(2 * m, 512, 0, 0), (2 * m + 1, 512, 0, 512)]
                        )
                    full_mask = False
                n_pairs = len(pairs)
                for h in range(GH):
                    pso = psoP.tile([128, 512], F32, tag="pso")
                    first_pv = True
                    for pi, pair in enumerate(pairs):
                        totw = sum(p[1] for p in pair)
                        pss = pssP.tile([128, 1024], F32, tag="pss")
                        for kt, w, qoff, poff in pair:
                            nc.tensor.matmul(
                                pss[:, poff : poff + w],
                                lhsT=KTzs[kt // 4][
                                    :, h, (kt % 4) * 128 : (kt % 4 + 1) * 128
                                ],
                                rhs=QT[:, h // 2, qt * 512 + qoff : qt * 512 + qoff + w],
                                start=True,
                                stop=True,
                            )
                        p_sb = cp.tile([128, 1024], F16, tag="p", bufs=5)
                        nc.scalar.activation(
                            p_sb[:, 0:totw],
                            pss[:, 0:totw],
                            mybir.ActivationFunctionType.Exp,
                            scale=0.125,
                        )
                        for kt, w, qoff, poff in pair:
                            if kt < 4 * qt:
                                continue  # off-diagonal: no mask
                            if full_mask:
                                s = 384 - 128 * kt
                                nc.vector.tensor_tensor(
                                    p_sb[:, poff : poff + w],
                                    p_sb[:, poff : poff + w],
                                    MASK[:, s : s + w],
                                    mybir.AluOpType.mult,
                                )
                            else:
                                # truncated: only first 128 cols can violate
                                nc.vector.tensor_tensor(
                                    p_sb[:, poff : poff + 128],
                                    p_sb[:, poff : poff + 128],
                                    MASK[:, 384:512],
                                    mybir.AluOpType.mult,
                                )
                        if pend_pv is not None:
                            emit_pv(pend_pv)
                            pop_deferred()
                        entries = []
                        for kt, w, qoff, poff in pair:
                            entries.append(
                                (
                                    kt,
                                    w,
                                    qoff,
                                    poff,
                                    first_pv,
                                    pi == n_pairs - 1 and kt == pair[-1][0],
                                )
                            )
                            first_pv = False
                        pend_pv = (pso, p_sb, entries, h, qt, pi == n_pairs - 1)
            emit_pv(pend_pv)
            while dnorm or dnorm_b or dfill or dproj:
                pop_deferred(tail=True)

    nc.compile()
    return nc


_NC_CACHE = None


def _get_program():
    global _NC_CACHE
    if _NC_CACHE is None:
        _NC_CACHE = _build_program()
    return _NC_CACHE


def _run(in_maps, trace=False, **kw):
    nc = _get_program()
    return run_bass_kernel_spmd(nc, in_maps, list(range(NCORES)), trace=trace, **kw)


def _t16(a):
    return np.ascontiguousarray(np.asarray(a).T.astype(np.float16, order="C"))


def _make_in_maps(x, Wq, bq, Wk, bk, Wv, bv, Wo, bo):
    xts = [_t16(np.asarray(x)[b]) for b in range(B)]  # [D, L] f16
    in_maps = []
    for core in range(NCORES):
        b, g = divmod(core, 4)
        s = slice(g * C, (g + 1) * C)
        in_maps.append(
            {
                "xt": xts[b],
                "wqt": _t16(np.asarray(Wq)[s, :]),  # [D, C]
                "wkt": _t16(np.asarray(Wk)[s, :]),
                "wvt": _t16(np.asarray(Wv)[s, :]),
                "wot": _t16(np.asarray(Wo)[:, s]),  # [C, D]
                "bq": np.ascontiguousarray(np.asarray(bq)[s], dtype=np.float32),
                "bk": np.ascontiguousarray(np.asarray(bk)[s], dtype=np.float32),
                "bv": np.tile(np.asarray(bv)[s].astype(np.float16), 4),
            }
        )
    return in_maps


def kernel(x, Wq, bq, Wk, bk, Wv, bv, Wo, bo, _trace=False, _trace_out=None, _tmpdir=None):
    in_maps = _make_in_maps(x, Wq, bq, Wk, bk, Wv, bv, Wo, bo)
    res = _run(in_maps, trace=_trace, tmpdir=_tmpdir)
    if _trace_out is not None:
        _trace_out.append(res)
    bo = np.asarray(bo, dtype=np.float32)
    out = np.empty((B, L, D), dtype=np.float32)
    for b in range(B):
        acc = res.results[4 * b]["out"].astype(np.float32)
        for g in range(1, 4):
            acc = acc + res.results[4 * b + g]["out"].astype(np.float32)
        out[b] = acc + bo[None, :]
    return out
